# revision 18
# baseline (speedup 1.0000x reference)
"""Trainium2 Bass kernel for multi-head attention (B=4, N=2048, DIM=1024, H=16, DH=64).

Sharding: 8 cores = 4 batches x 2 query-halves. Each core receives x[b]^T with
its query-half columns rotated to the front (attention is invariant to a
consistent permutation of the key/value axis), computes q for columns 0:1024,
k/v for all 2048, runs scores^T = k_h^T @ q_h per head (row-tiled pairs),
softmax via exp + ones-column denominator folded into the AV matmul, and the
full output projection for its rows. Outputs are disjoint across cores.
"""

import os

import numpy as np
import ml_dtypes

import concourse.bass as bass
import concourse.tile as tile
from concourse import bacc, mybir
from concourse import bass_utils

B, N, DIM = 4, 2048, 1024
HEADS, DH = 16, 64
INNER = HEADS * DH
SCALE = DH ** -0.5
NCORES = 8
IH = N // 2          # query rows per core (i-half)
BF16 = mybir.dt.bfloat16
F32 = mybir.dt.float32

KT = DIM // 128          # 8 contraction tiles for projections
NT = N // 128            # 16 j tiles
ES = INNER // 128        # 8 e-slices for q or k

_CACHE = {}


def _build_program():
    nc = bacc.Bacc("TRN2", target_bir_lowering=False, debug=False)

    xT_d = nc.dram_tensor("xT", [DIM, N], BF16, kind="ExternalInput")
    wqkv_d = nc.dram_tensor("w_qkv", [DIM, 3 * INNER], BF16, kind="ExternalInput")
    wout_d = nc.dram_tensor("w_out", [INNER, DIM], BF16, kind="ExternalInput")
    bout_d = nc.dram_tensor("b_out", [DIM], F32, kind="ExternalInput")
    out_d = nc.dram_tensor("out", [IH, DIM], F32, kind="ExternalOutput")

    with tile.TileContext(nc) as tc:
        _emit(tc, nc, xT_d, wqkv_d, wout_d, bout_d, out_d)
    nc.compile()
    return nc


def _emit(tc, nc, xT_d, wqkv_d, wout_d, bout_d, out_d):
    from contextlib import ExitStack

    xT_r = xT_d.ap().rearrange("(t p) n -> p t n", p=128)       # [128, 8, 2048]
    w_r = wqkv_d.ap().rearrange("(t p) e -> p t e", p=128)      # [128, 8, 3072]
    wo_r = wout_d.ap().rearrange("(t p) d -> p t d", p=128)     # [128, 8, 1024]

    bap = bout_d.ap()
    bias_bcast = bass.AP(tensor=bap.tensor, offset=bap.offset,
                         ap=[[0, 128]] + [list(d) for d in bap.ap])

    with ExitStack() as ctx:
        consts = ctx.enter_context(tc.tile_pool(name="consts", bufs=1))
        qkv_out = ctx.enter_context(tc.tile_pool(name="qkv_out", bufs=1))
        attn_out = ctx.enter_context(tc.tile_pool(name="attn_out", bufs=1))
        atp = ctx.enter_context(tc.tile_pool(name="attnT", bufs=4))
        rcp = ctx.enter_context(tc.tile_pool(name="rcp", bufs=1))
        bcsp = ctx.enter_context(tc.tile_pool(name="bcs", bufs=2))
        avup = ctx.enter_context(tc.tile_pool(name="avu", bufs=2))
        oddp = ctx.enter_context(tc.tile_pool(name="odd", bufs=1))
        drbp = ctx.enter_context(tc.tile_pool(name="drb", bufs=2, space="DRAM"))
        ps_sc = ctx.enter_context(tc.tile_pool(name="ps_sc", bufs=2, space="PSUM"))

        bias_sb = consts.tile([128, DIM], F32)
        nc.sync.dma_start(out=bias_sb, in_=bias_bcast)
        wo_sb = consts.tile([128, ES, DIM], BF16)       # head pair hp at [:, hp, :]
        nc.sync.dma_start(out=wo_sb, in_=wo_r)

        qTs = [qkv_out.tile([128, IH], BF16, name=f"qT{s}") for s in range(ES)]
        kTs = [qkv_out.tile([128, N], BF16, name=f"kT{s}") for s in range(ES)]
        v_lo = qkv_out.tile([128, NT, 8, DH + 1], BF16)  # heads 0-7 (+ones col)
        v_hi = qkv_out.tile([128, NT, 8, DH + 1], BF16)  # heads 8-15
        nc.vector.memset(v_lo[:, :, :, DH], 1.0)
        nc.vector.memset(v_hi[:, :, :, DH], 1.0)
        aoTs = [attn_out.tile([128, IH], BF16, name=f"aoT{s}") for s in range(ES)]

        # ---------------- phase 1: projections ----------------
        # group order: v_lo, k(s0-3), q(s0-3) -> enables head pairs 0-3;
        # then v_hi, k(s4-7), q(s4-7) -> pairs 4-7.
        with tc.tile_pool(name="p1_x", bufs=1) as p1x, \
             tc.tile_pool(name="p1_w", bufs=2) as p1w, \
             tc.tile_pool(name="p1_ps", bufs=2, space="PSUM") as p1ps:
            xTk = [p1x.tile([128, N], BF16, name=f"xTk{k}") for k in range(KT)]
            for k in range(KT):
                nc.sync.dma_start(out=xTk[k], in_=xT_r[:, k, :])

            for g in (4, 2, 0, 5, 3, 1):            # e-groups of 512 cols
                wg = p1w.tile([128, KT, 512], BF16, tag="wg")
                nc.sync.dma_start(out=wg, in_=w_r[:, :, 512 * g:512 * (g + 1)])
                if g < 2:
                    # q columns: qT e-slices 4g..4g+3 (i = cols 0:IH of xT)
                    for s4 in range(4):
                        s = 4 * g + s4
                        ps = p1ps.tile([128, IH], F32, tag="ps")
                        for c in range(IH // 512):
                            for k in range(KT):
                                nc.tensor.matmul(
                                    ps[:, 512 * c:512 * (c + 1)],
                                    wg[:, k, 128 * s4:128 * (s4 + 1)],
                                    xTk[k][:, 512 * c:512 * (c + 1)],
                                    start=(k == 0), stop=(k == KT - 1))
                        nc.vector.tensor_copy(out=qTs[s], in_=ps)
                elif g < 4:
                    # k columns: kT e-slices 4(g-2)..+3
                    for s4 in range(4):
                        s = 4 * (g - 2) + s4
                        for half in range(2):
                            ps = p1ps.tile([128, IH], F32, tag="ps")
                            for c in range(IH // 512):
                                for k in range(KT):
                                    nc.tensor.matmul(
                                        ps[:, 512 * c:512 * (c + 1)],
                                        wg[:, k, 128 * s4:128 * (s4 + 1)],
                                        xTk[k][:, IH * half + 512 * c:IH * half + 512 * (c + 1)],
                                        start=(k == 0), stop=(k == KT - 1))
                            nc.vector.tensor_copy(
                                out=kTs[s][:, IH * half:IH * (half + 1)], in_=ps)
                else:
                    # v columns: heads 8*(g-4) .. +8 ; out v[n, e]
                    vdst = v_lo if g == 4 else v_hi
                    for t in range(NT):
                        ps = p1ps.tile([128, 512], F32, tag="ps")
                        for k in range(KT):
                            nc.tensor.matmul(
                                ps, xTk[k][:, 128 * t:128 * (t + 1)],
                                wg[:, k, :],
                                start=(k == 0), stop=(k == KT - 1))
                        nc.vector.tensor_copy(
                            out=vdst[:, t, :, 0:DH],
                            in_=ps.rearrange("p (h d) -> p h d", h=8))

        # ---------------- phase 2: attention (+ phase 3 weaved at the end) ----
        with tc.tile_pool(name="ps_av", bufs=2, space="PSUM") as ps_av, \
             tc.tile_pool(name="p3_st", bufs=2) as p3st:
            for s in range(ES):                 # head pair (2s, 2s+1)
                av0 = ps_av.tile([DH + 1, IH], F32, tag="av")
                av1 = ps_av.tile([DH + 1, IH], F32, tag="av")
                avs = [av0, av1]
                for t in range(NT):
                    for p in range(2):          # head half within pair
                        h = 2 * s + p
                        pb = 64 * p
                        sc = ps_sc.tile([128, IH], F32, tag="sc")
                        for c in range(IH // 512):
                            nc.tensor.matmul(
                                sc[:, 512 * c:512 * (c + 1)],
                                kTs[s][pb:pb + 64, 128 * t:128 * (t + 1)],
                                qTs[s][pb:pb + 64, 512 * c:512 * (c + 1)],
                                start=True, stop=True,
                                tile_position=(pb, 0))
                        at = atp.tile([128, IH], BF16, tag="at")
                        nc.scalar.activation(
                            out=at, in_=sc,
                            func=mybir.ActivationFunctionType.Exp, scale=SCALE)
                        vsrc = v_lo if h < 8 else v_hi
                        for c in range(IH // 512):
                            nc.tensor.matmul(
                                avs[p][:, 512 * c:512 * (c + 1)],
                                vsrc[:, t, h % 8, :],
                                at[:, 512 * c:512 * (c + 1)],
                                start=(t == 0), stop=(t == NT - 1))
                for p in range(2):
                    av = avs[p]
                    avu = avup.tile([DH + 1, IH], F32, tag="avu")
                    nc.vector.tensor_copy(out=avu, in_=av)
                    rc = rcp.tile([128, IH], BF16, tag="rc")
                    with nc.allow_low_precision(reason="softmax denom recip in bf16"):
                        nc.vector.reciprocal(
                            out=rc[DH:DH + 1, :], in_=avu[DH:DH + 1, :])
                    dr = drbp.tile([IH], BF16, tag="dr")
                    nc.sync.dma_start(out=dr, in_=rc[DH:DH + 1, :])
                    dr_bc = bass.AP(tensor=dr.tensor, offset=dr.offset,
                                    ap=[[0, DH]] + [list(dd) for dd in dr.ap])
                    bcs = bcsp.tile([DH, IH], BF16, tag="bcs")
                    nc.sync.dma_start(out=bcs, in_=dr_bc)
                    if p == 0:
                        nc.vector.tensor_mul(
                            out=aoTs[s][0:DH, :], in0=avu[0:DH, :], in1=bcs)
                    else:
                        od = oddp.tile([DH, IH], BF16, tag="od")
                        nc.vector.tensor_mul(out=od, in0=avu[0:DH, :], in1=bcs)
                        nc.sync.dma_start(out=aoTs[s][DH:128, :], in_=od)

            # ---------------- phase 3: output projection ----------------
            for ns in range(IH // 128):
                po = ps_av.tile([128, DIM], F32, tag="av", name=f"po{ns}")
                for c in range(DIM // 512):
                    for hp in range(ES):
                        nc.tensor.matmul(
                            po[:, 512 * c:512 * (c + 1)],
                            aoTs[hp][:, 128 * ns:128 * (ns + 1)],
                            wo_sb[:, hp, 512 * c:512 * (c + 1)],
                            start=(hp == 0), stop=(hp == ES - 1))
                st = p3st.tile([128, DIM], F32, tag="st")
                nc.vector.tensor_add(out=st, in0=po, in1=bias_sb)
                nc.sync.dma_start(out=out_d.ap()[128 * ns:128 * (ns + 1), :], in_=st)


def get_program():
    if "nc" not in _CACHE:
        _CACHE["nc"] = _build_program()
    return _CACHE["nc"]


def make_in_maps(x, w_qkv, w_out, b_out):
    bf = ml_dtypes.bfloat16
    w_qkv_b = np.ascontiguousarray(w_qkv, np.float32).astype(bf)
    w_out_b = np.ascontiguousarray(w_out, np.float32).astype(bf)
    b_out_f = np.ascontiguousarray(b_out, np.float32)
    in_maps = []
    for core in range(NCORES):
        b, half = core // 2, core % 2
        xT = np.ascontiguousarray(np.asarray(x[b], np.float32).T).astype(bf)
        if half == 1:   # rotate this core's query half to the front
            xT = np.concatenate([xT[:, IH:], xT[:, :IH]], axis=1)
        in_maps.append({
            "xT": np.ascontiguousarray(xT),
            "w_qkv": w_qkv_b,
            "w_out": w_out_b,
            "b_out": b_out_f,
        })
    return in_maps


def kernel(x, w_qkv, w_out, b_out):
    nc = get_program()
    in_maps = make_in_maps(x, w_qkv, w_out, b_out)
    res = bass_utils.run_bass_kernel_spmd(nc, in_maps, core_ids=list(range(NCORES)))
    out = np.empty((B, N, DIM), np.float32)
    for core in range(NCORES):
        b, half = core // 2, core % 2
        out[b, IH * half:IH * (half + 1), :] = res.results[core]["out"]
    return out
